# revision 54
# baseline (speedup 1.0000x reference)
"""Causal attention (B=4, S=2048, D=1024, single head) on 8 TRN2 NeuronCores.

Sharding: data-parallel over batch x causal-balanced query split.
  core c -> batch b = c//2, role r = c%2.
  Queries: the 8 tiles of 256 rows have causal visit-needs
  [1,1,2,2,3,3,4,4] key blocks (of 512). Role 0 takes tiles {0,3,4,7},
  role 1 takes {1,2,5,6}: both multisets of needs are {1,2,3,4}, so one
  SPMD program with per-slot visit counts (1,2,3,4) has zero padding and
  both cores do identical work.
  K/V: each core projects only its half of the sequence (role 0 rows
  0:1024, role 1 rows 1024:2048) and the halves are exchanged pair-wise
  with AllGather collectives (replica groups {2b, 2b+1}), split in two
  chunks each so attention can start on early key blocks.

Per-core differences (which query rows, which keys are causally visible)
are carried in input data only: xqt/xth are host-sliced columns of x^T,
qidx holds each local query row's global index, and causality is a
data-driven additive mask (-1e6 where kpos > qidx) on the DVE.

Compute is bf16 on the TensorEngine with f32 PSUM accumulation; softmax
skips the running max (logits are ~N(0,1) after the 1/32 scale; masked
lanes sit at -31250 and underflow to exactly 0).
"""

import sys

if "/opt/trn_rl_repo" not in sys.path:
    sys.path.insert(0, "/opt/trn_rl_repo")

import ml_dtypes
import numpy as np

import bass_rust

import concourse.bass as bass
import concourse.mybir as mybir
from concourse.masks import make_identity
from concourse.tile import TileContext
from concourse.tile_rust import add_dep_helper

B, S, D = 4, 2048, 1024
P = 128
NCORES = 8
DC = D // P           # 8 contraction chunks of 128
QROWS = S // 2        # 1024 query rows per core
QT = QROWS // P       # 8 query tiles of 128 rows
SH = S // 2           # this core's K/V half
KBLK = 512            # key block size
NKB = S // KBLK       # 4 key blocks
SCALE = 1.0 / np.sqrt(np.float32(D))
MASK_NEG = -1.0e6
GROUPS = [[0, 1], [2, 3], [4, 5], [6, 7]]

F32 = mybir.dt.float32
BF16 = mybir.dt.bfloat16


# ---------------------------------------------------------------------------
# This container's walrus build (setupSyncWait, CoreV2/V3GenImpl.cpp) rejects
# any instruction carrying more than one sem wait. Tile's wait-assignment
# freely emits several. Hoist all but one wait of each instruction onto NOPs
# inserted immediately before it on the same engine — the engine executes its
# stream in order, so waiting on a preceding same-engine NOP is equivalent.
def _split_multi_waits(nc):
    n_split = 0
    for fn in nc.m.functions:
        for bb in fn.blocks:
            insts = list(bb.instructions)
            out = []
            changed = False
            for inst in insts:
                si = inst.sync_info
                if si is not None and len(si.on_wait) > 1:
                    waits = list(si.on_wait)
                    for w in waits[:-1]:
                        nop = mybir.InstNoOp(
                            name=f"{inst.name}-wsplit{n_split}", ins=[], outs=[]
                        )
                        n_split += 1
                        nop.engine = inst.engine
                        nop.sync_info = bass_rust.SyncInfo(
                            on_wait=[w], on_update=[]
                        )
                        out.append(nop)
                    inst.sync_info = bass_rust.SyncInfo(
                        on_wait=[waits[-1]], on_update=list(si.on_update)
                    )
                    changed = True
                if si is not None and len(si.on_update) > 2:
                    raise RuntimeError(
                        f"{inst.name}: {len(si.on_update)} sync updates; "
                        "update-splitting not implemented"
                    )
                out.append(inst)
            if changed:
                bb.instructions = out
    return nc
# ---------------------------------------------------------------------------


def _build_nc():
    nc = bass.Bass()

    xth = nc.declare_dram_parameter("xth", [D, SH], BF16, isOutput=False)
    xqt = nc.declare_dram_parameter("xqt", [D, QROWS], BF16, isOutput=False)
    wq = nc.declare_dram_parameter("wq", [D, D], BF16, isOutput=False)
    wk = nc.declare_dram_parameter("wk", [D, D], BF16, isOutput=False)
    wv = nc.declare_dram_parameter("wv", [D, D], BF16, isOutput=False)
    qidx = nc.declare_dram_parameter("qidx", [QROWS], F32, isOutput=False)
    rk = nc.declare_dram_parameter("rk", [1, 1], mybir.dt.uint32, isOutput=False)
    out = nc.declare_dram_parameter("out", [QROWS, D], F32, isOutput=True)

    xth_r = xth.rearrange("(dc p) s -> p dc s", p=P)
    xqt_r = xqt.rearrange("(dc p) s -> p dc s", p=P)
    wq_r = wq.rearrange("(dc p) e -> p dc e", p=P)
    wk_r = wk.rearrange("(dc p) e -> p dc e", p=P)
    wv_r = wv.rearrange("(dc p) e -> p dc e", p=P)
    qidx_r = qidx.rearrange("(t p) -> p t", p=P)

    with TileContext(nc) as tc:
        # The race-detector sim can't model pair-aliased Shared DRAM (it
        # demands a single writer); ordering for the shared exchange is
        # enforced with explicit deps instead.
        tc.race_detector_enabled = False

        # Long-lived tiles. K^T / V are per-key-block so attention only
        # waits on the specific block's collective, not the whole tensor.
        persist = tc.alloc_tile_pool(name="persist", bufs=1)
        qt_sb = persist.tile([P, DC, QROWS], BF16, tag="qt_sb")   # Q^T [e, q]
        kt_b = [
            persist.tile([P, DC, KBLK], BF16, tag=f"kt_b{v}", name=f"kt_b{v}")
            for v in range(NKB)
        ]
        v_b = [
            persist.tile([P, KBLK // P, D], BF16, tag=f"v_b{v}", name=f"v_b{v}")
            for v in range(NKB)
        ]
        kpos_f = persist.tile([P, S], F32, tag="kpos_f")
        qidx_sb = persist.tile([P, QT], F32, tag="qidx_sb")
        ident = persist.tile([P, P], BF16, tag="ident")

        nc.sync.dma_start(qidx_sb[:], qidx_r)
        make_identity(nc, ident[:])

        # ---- Phase 1: projections + pair-wise K/V exchange ----
        with (
            tc.tile_pool(name="proj_in", bufs=1) as proj_in,
            tc.tile_pool(name="proj_w", bufs=2) as proj_w,
            tc.tile_pool(name="proj_st", bufs=2) as proj_st,
            tc.tile_pool(name="proj_ps", bufs=6, space="PSUM") as proj_ps,
            tc.tile_pool(name="cc_dram", bufs=1, space="DRAM") as cc_dram,
        ):
            # Dependency-free throwaway collective, emitted before anything
            # else: absorbs the ~50us (jittery) ncfw boot so the rendezvous
            # barriers below run at their ~5us post-boot cost. Collectives
            # also have a ~23us minimum spacing on this firmware, so the
            # earlier this runs the earlier the real barriers may run.
            # Barrier collectives gather garbage — no input producer needed.
            wm_in = cc_dram.tile([16], F32, tag="wm_in")
            wm_out = cc_dram.tile([2, 16], F32, tag="wm_out")
            nc.gpsimd.collective_compute(
                "AllGather",
                mybir.AluOpType.bypass,
                replica_groups=GROUPS,
                ins=[wm_in[:]],
                outs=[wm_out[:]],
            )

            xth_sb = proj_in.tile([P, DC, SH], BF16, tag="xth_sb")
            xqt_sb = proj_in.tile([P, DC, QROWS], BF16, tag="xqt_sb")

            # iota values < 2048 are exact in f32
            nc.gpsimd.iota(
                kpos_f[:], pattern=[[1, S]], base=0, channel_multiplier=0,
                allow_small_or_imprecise_dtypes=True,
            )

            # All four K/V half-exchanges fit under PE cover when launched
            # in consumption order (KT0, KT1, V0, V1 — V blocks are consumed
            # latest). DMA order = first-use order, with the first-needed
            # tensors split fine so all 16 DMA queues fill immediately.
            # The very first matmul group (KTh0, et=0) needs only wk columns
            # 0:128 plus xth half 0 — ship exactly that first, spread over
            # many queues, so the TensorEngine starts as early as possible.
            wk_sb = proj_w.tile([P, DC, D], BF16, tag="w", bufs=3)
            for dc in range(0, DC, 2):
                nc.sync.dma_start(
                    wk_sb[:, dc : dc + 2, 0:P], wk_r[:, dc : dc + 2, 0:P]
                )
            for dc in range(DC):
                nc.sync.dma_start(
                    xth_sb[:, dc, 0:KBLK], xth_r[:, dc, 0:KBLK]
                )
            for et in range(1, 8):
                esl = slice(et * P, (et + 1) * P)
                for dc in range(0, DC, 4):
                    nc.sync.dma_start(
                        wk_sb[:, dc : dc + 4, esl], wk_r[:, dc : dc + 4, esl]
                    )
            for dc in range(0, DC, 2):
                nc.sync.dma_start(
                    xth_sb[:, dc : dc + 2, KBLK:SH], xth_r[:, dc : dc + 2, KBLK:SH]
                )
            wv_sb = proj_w.tile([P, DC, D], BF16, tag="w", bufs=3)
            for dc in range(0, DC, 2):
                nc.sync.dma_start(wv_sb[:, dc : dc + 2, :], wv_r[:, dc : dc + 2, :])
            wq_sb = proj_w.tile([P, DC, D], BF16, tag="w", bufs=3)
            for dc in range(0, DC, 2):
                nc.sync.dma_start(wq_sb[:, dc : dc + 2, :], wq_r[:, dc : dc + 2, :])
            for dc in range(0, DC, 2):
                nc.sync.dma_start(xqt_sb[:, dc : dc + 2, :], xqt_r[:, dc : dc + 2, :])

            # K^T/V halves are exchanged through pair-shared DRAM (cores
            # 2k/2k+1 alias addr_space="Shared" allocations): each core
            # DMA-writes its stagings into its rank's slot (runtime branch
            # on the rank register — the only non-data-driven role split),
            # one tiny AllGather acts as the pair rendezvous, then both
            # halves are DMA-read back at full bandwidth. This replaces 4
            # slow data collectives (~20us/MB) with plain DMA.
            def v_half(h):
                vst = proj_st.tile(
                    [P, KBLK // P, D], BF16, tag=f"vst{h}", name=f"vst{h}", bufs=1
                )
                for st in range(KBLK // P):
                    for ec in range(D // KBLK):
                        ps = proj_ps.tile([P, KBLK], F32, tag="proj_ps")
                        for dc in range(DC):
                            nc.tensor.matmul(
                                ps[:],
                                xth_sb[:, dc, h * KBLK + st * P : h * KBLK + (st + 1) * P],
                                wv_sb[:, dc, ec * KBLK : (ec + 1) * KBLK],
                                start=(dc == 0),
                                stop=(dc == DC - 1),
                            )
                        nc.scalar.copy(vst[:, st, ec * KBLK : (ec + 1) * KBLK], ps[:])
                return vst

            def kt_half(h):
                ssl = slice(h * KBLK, (h + 1) * KBLK)
                ktst = proj_st.tile(
                    [P, DC, KBLK], BF16, tag=f"ktst{h}", name=f"ktst{h}", bufs=1
                )
                for et in range(DC):
                    ps = proj_ps.tile([P, KBLK], F32, tag="proj_ps")
                    for dc in range(DC):
                        nc.tensor.matmul(
                            ps[:],
                            wk_sb[:, dc, et * P : (et + 1) * P],
                            xth_sb[:, dc, ssl],
                            start=(dc == 0),
                            stop=(dc == DC - 1),
                        )
                    nc.scalar.copy(ktst[:, et, :], ps[:])
                return ktst

            # One Shared tensor per (rank, slot) — the scheduler sim demands
            # a single writer inst per Shared DRAM tensor. Slots: 0=KTh0,
            # 1=Vh0, 2=KTh1, 3=Vh1 (flat 512K bf16 each).
            sh_d = [
                [
                    cc_dram.tile(
                        [D * KBLK], BF16, tag=f"sh_d{r}{j}",
                        name=f"sh_d{r}{j}", addr_space="Shared",
                    )
                    for j in range(4)
                ]
                for r in range(2)
            ]

            def kt_view(flat):
                return flat.rearrange("(et p s) -> p et s", p=P, s=KBLK)

            def v_view(flat):
                return flat.rearrange("(st p e) -> p st e", p=P, e=D)

            rk_reg = nc.sync.alloc_register("rk_reg")
            nc.sync.reg_load(rk_reg, rk[0:1, 0:1])

            def exchange_half(h, ktst, vst):
                # Write my stagings into my rank's pair-shared slots, tiny
                # AllGather as the pair rendezvous, read back both ranks'.
                writes = []
                with tc.If(nc.sync.snap(rk_reg) == 0) as cmp:
                    writes.append(
                        nc.sync.dma_start(kt_view(sh_d[0][2 * h]), ktst[:])
                    )
                    writes.append(
                        nc.sync.dma_start(v_view(sh_d[0][2 * h + 1]), vst[:])
                    )
                with cmp.Else():
                    writes.append(
                        nc.sync.dma_start(kt_view(sh_d[1][2 * h]), ktst[:])
                    )
                    writes.append(
                        nc.sync.dma_start(v_view(sh_d[1][2 * h + 1]), vst[:])
                    )
                b_in = cc_dram.tile([16], F32, tag=f"b_in{h}", name=f"b_in{h}")
                b_out = cc_dram.tile(
                    [2, 16], F32, tag=f"b_out{h}", name=f"b_out{h}"
                )
                cc = nc.gpsimd.collective_compute(
                    "AllGather",
                    mybir.AluOpType.bypass,
                    replica_groups=GROUPS,
                    ins=[b_in[:]],
                    outs=[b_out[:]],
                )
                for w in writes:
                    add_dep_helper(cc.ins, w.ins, True, "barrier after writes")
                for rank in range(2):
                    rd_k = nc.sync.dma_start(
                        kt_b[2 * rank + h][:], kt_view(sh_d[rank][2 * h])
                    )
                    rd_v = nc.sync.dma_start(
                        v_b[2 * rank + h][:], v_view(sh_d[rank][2 * h + 1])
                    )
                    add_dep_helper(rd_k.ins, cc.ins, True, "read after rdv")
                    add_dep_helper(rd_v.ins, cc.ins, True, "read after rdv")

            ktst0 = kt_half(0)
            vst0 = v_half(0)
            exchange_half(0, ktst0, vst0)
            ktst1 = kt_half(1)
            vst1 = v_half(1)
            exchange_half(1, ktst1, vst1)

            # Q^T [e, q] = Wq^T @ xq^T (overlaps the second collective).
            for et in range(DC):
                for sc in range(QROWS // KBLK):
                    ps = proj_ps.tile([P, KBLK], F32, tag="proj_ps")
                    for dc in range(DC):
                        nc.tensor.matmul(
                            ps[:],
                            wq_sb[:, dc, et * P : (et + 1) * P],
                            xqt_sb[:, dc, sc * KBLK : (sc + 1) * KBLK],
                            start=(dc == 0),
                            stop=(dc == DC - 1),
                        )
                    nc.scalar.copy(qt_sb[:, et, sc * KBLK : (sc + 1) * KBLK], ps[:])

        # ---- Phase 2: block attention ----
        with (
            tc.tile_pool(name="att", bufs=2) as att,
            tc.tile_pool(name="att_sm", bufs=3) as att_sm,
            tc.tile_pool(name="ps_sc", bufs=2, space="PSUM") as ps_sc,
            tc.tile_pool(name="ps_pt", bufs=2, space="PSUM") as ps_pt,
            tc.tile_pool(name="ps_ctx", bufs=2, space="PSUM") as ps_ctx,
        ):
            for qt in range(QT):
                # 256-row slot s = qt//2 visits s+1 key blocks.
                nvis = qt // 2 + 1
                nkc = nvis * (KBLK // P)
                p_sb = att.tile([P, S], BF16, tag="p_sb")
                pt_sb = att.tile([P, S // P, P], BF16, tag="pt_sb")
                sums = att_sm.tile([P, NKB], F32, tag="sums")
                qcol = qidx_sb[:, qt : qt + 1]

                for v in range(nvis):
                    ksl = slice(v * KBLK, (v + 1) * KBLK)
                    sc_ps = ps_sc.tile([P, KBLK], F32, tag="sc_ps")
                    for ec in range(DC):
                        nc.tensor.matmul(
                            sc_ps[:],
                            qt_sb[:, ec, qt * P : (qt + 1) * P],
                            kt_b[v][:, ec, :],
                            start=(ec == 0),
                            stop=(ec == DC - 1),
                        )
                    bias = att_sm.tile([P, KBLK], F32, tag="bias")
                    nc.vector.tensor_scalar(
                        bias[:], kpos_f[:, ksl], qcol, MASK_NEG,
                        mybir.AluOpType.is_gt, mybir.AluOpType.mult,
                    )
                    sm = att_sm.tile([P, KBLK], F32, tag="sm")
                    nc.vector.tensor_add(sm[:], sc_ps[:], bias[:])
                    nc.scalar.activation(
                        p_sb[:, ksl], sm[:],
                        mybir.ActivationFunctionType.Exp,
                        scale=float(SCALE),
                        accum_out=sums[:, v : v + 1],
                    )

                for kc in range(nkc):
                    pt_ps = ps_pt.tile([P, P], BF16, tag="pt_ps")
                    nc.tensor.transpose(
                        pt_ps[:], p_sb[:, kc * P : (kc + 1) * P], ident[:]
                    )
                    nc.vector.tensor_copy(pt_sb[:, kc, :], pt_ps[:])

                tot = att_sm.tile([P, 1], F32, tag="tot")
                rinv = att_sm.tile([P, 1], F32, tag="rinv")
                nc.vector.reduce_sum(
                    tot[:], sums[:, :nvis], axis=mybir.AxisListType.X
                )
                nc.vector.reciprocal(rinv[:], tot[:])

                ctx_lo = ps_ctx.tile([P, KBLK], F32, tag="ctx_lo")
                ctx_hi = ps_ctx.tile([P, KBLK], F32, tag="ctx_hi")
                for kc in range(nkc):
                    vb = v_b[kc // (KBLK // P)]
                    vrow = kc % (KBLK // P)
                    nc.tensor.matmul(
                        ctx_lo[:], pt_sb[:, kc, :], vb[:, vrow, 0:KBLK],
                        start=(kc == 0), stop=(kc == nkc - 1),
                    )
                    nc.tensor.matmul(
                        ctx_hi[:], pt_sb[:, kc, :], vb[:, vrow, KBLK:D],
                        start=(kc == 0), stop=(kc == nkc - 1),
                    )

                out_sb = att.tile([P, D], F32, tag="out_sb")
                nc.vector.tensor_scalar_mul(out_sb[:, 0:KBLK], ctx_lo[:], rinv[:])
                nc.vector.tensor_scalar_mul(out_sb[:, KBLK:D], ctx_hi[:], rinv[:])
                nc.sync.dma_start(out[qt * P : (qt + 1) * P, :], out_sb[:])

        persist.release()

    return _split_multi_waits(nc)


_NC_CACHE = None


def _get_nc():
    global _NC_CACHE
    if _NC_CACHE is None:
        _NC_CACHE = _build_nc()
    return _NC_CACHE


_TILE256 = {0: (0, 3, 4, 7), 1: (1, 2, 5, 6)}


def _qrows(role):
    # 256-row tiles ordered by ascending visit-need (1,2,3,4 key blocks).
    return np.concatenate(
        [np.arange(t * 256, (t + 1) * 256) for t in _TILE256[role]]
    )


def _shard_inputs(x, Wq, Wk, Wv):
    bf = ml_dtypes.bfloat16
    w = {
        "wq": np.ascontiguousarray(Wq.astype(bf)),
        "wk": np.ascontiguousarray(Wk.astype(bf)),
        "wv": np.ascontiguousarray(Wv.astype(bf)),
    }
    in_maps = []
    for c in range(NCORES):
        b, r = c // 2, c % 2
        rows = _qrows(r)
        xbT = x[b].T.astype(bf)                                  # [D, S]
        in_maps.append(
            {
                "xth": np.ascontiguousarray(xbT[:, r * SH : (r + 1) * SH]),
                "xqt": np.ascontiguousarray(xbT[:, rows]),
                "qidx": rows.astype(np.float32),
                "rk": np.array([[r]], dtype=np.uint32),
                **w,
            }
        )
    return in_maps


def _unshard(results, dtype):
    out = np.empty((B, S, D), dtype=dtype)
    for c in range(NCORES):
        b, r = c // 2, c % 2
        out[b, _qrows(r), :] = results[c]["out"]
    return out


def run(x, Wq, Wk, Wv, trace=False, tmpdir=None):
    from concourse.bass_utils import run_bass_kernel_spmd

    nc = _get_nc()
    in_maps = _shard_inputs(x, Wq, Wk, Wv)
    res = run_bass_kernel_spmd(
        nc, in_maps, core_ids=list(range(NCORES)), trace=trace, tmpdir=tmpdir
    )
    return _unshard(res.results, np.dtype(x.dtype)), res


def kernel(x, Wq, Wk, Wv):
    out, _ = run(np.asarray(x), np.asarray(Wq), np.asarray(Wk), np.asarray(Wv))
    return out


# revision 55
# speedup vs baseline: 1.0105x; 1.0105x over previous
"""Causal attention (B=4, S=2048, D=1024, single head) on 8 TRN2 NeuronCores.

Sharding: data-parallel over batch x causal-balanced query split.
  core c -> batch b = c//2, role r = c%2.
  Queries: the 8 tiles of 256 rows have causal visit-needs
  [1,1,2,2,3,3,4,4] key blocks (of 512). Role 0 takes tiles {0,3,4,7},
  role 1 takes {1,2,5,6}: both multisets of needs are {1,2,3,4}, so one
  SPMD program with per-slot visit counts (1,2,3,4) has zero padding and
  both cores do identical work.
  K/V: each core projects only its half of the sequence (role 0 rows
  0:1024, role 1 rows 1024:2048) and the halves are exchanged pair-wise
  with AllGather collectives (replica groups {2b, 2b+1}), split in two
  chunks each so attention can start on early key blocks.

Per-core differences (which query rows, which keys are causally visible)
are carried in input data only: xqt/xth are host-sliced columns of x^T,
qidx holds each local query row's global index, and causality is a
data-driven additive mask (-1e6 where kpos > qidx) on the DVE.

Compute is bf16 on the TensorEngine with f32 PSUM accumulation; softmax
skips the running max (logits are ~N(0,1) after the 1/32 scale; masked
lanes sit at -31250 and underflow to exactly 0).
"""

import sys

if "/opt/trn_rl_repo" not in sys.path:
    sys.path.insert(0, "/opt/trn_rl_repo")

import ml_dtypes
import numpy as np

import bass_rust

import concourse.bass as bass
import concourse.mybir as mybir
from concourse.masks import make_identity
from concourse.tile import TileContext
from concourse.tile_rust import add_dep_helper

B, S, D = 4, 2048, 1024
P = 128
NCORES = 8
DC = D // P           # 8 contraction chunks of 128
QROWS = S // 2        # 1024 query rows per core
QT = QROWS // P       # 8 query tiles of 128 rows
SH = S // 2           # this core's K/V half
KBLK = 512            # key block size
NKB = S // KBLK       # 4 key blocks
SCALE = 1.0 / np.sqrt(np.float32(D))
MASK_NEG = -1.0e6
GROUPS = [[0, 1], [2, 3], [4, 5], [6, 7]]

F32 = mybir.dt.float32
BF16 = mybir.dt.bfloat16


# ---------------------------------------------------------------------------
# This container's walrus build (setupSyncWait, CoreV2/V3GenImpl.cpp) rejects
# any instruction carrying more than one sem wait. Tile's wait-assignment
# freely emits several. Hoist all but one wait of each instruction onto NOPs
# inserted immediately before it on the same engine — the engine executes its
# stream in order, so waiting on a preceding same-engine NOP is equivalent.
def _split_multi_waits(nc):
    n_split = 0
    for fn in nc.m.functions:
        for bb in fn.blocks:
            insts = list(bb.instructions)
            out = []
            changed = False
            for inst in insts:
                si = inst.sync_info
                if si is not None and len(si.on_wait) > 1:
                    waits = list(si.on_wait)
                    for w in waits[:-1]:
                        nop = mybir.InstNoOp(
                            name=f"{inst.name}-wsplit{n_split}", ins=[], outs=[]
                        )
                        n_split += 1
                        nop.engine = inst.engine
                        nop.sync_info = bass_rust.SyncInfo(
                            on_wait=[w], on_update=[]
                        )
                        out.append(nop)
                    inst.sync_info = bass_rust.SyncInfo(
                        on_wait=[waits[-1]], on_update=list(si.on_update)
                    )
                    changed = True
                if si is not None and len(si.on_update) > 2:
                    raise RuntimeError(
                        f"{inst.name}: {len(si.on_update)} sync updates; "
                        "update-splitting not implemented"
                    )
                out.append(inst)
            if changed:
                bb.instructions = out
    return nc
# ---------------------------------------------------------------------------


def _build_nc():
    nc = bass.Bass()

    xth = nc.declare_dram_parameter("xth", [D, SH], BF16, isOutput=False)
    xqt = nc.declare_dram_parameter("xqt", [D, QROWS], BF16, isOutput=False)
    wq = nc.declare_dram_parameter("wq", [D, D], BF16, isOutput=False)
    wk = nc.declare_dram_parameter("wk", [D, D], BF16, isOutput=False)
    wv = nc.declare_dram_parameter("wv", [D, D], BF16, isOutput=False)
    qidx = nc.declare_dram_parameter("qidx", [QROWS], F32, isOutput=False)
    rk = nc.declare_dram_parameter("rk", [1, 1], mybir.dt.uint32, isOutput=False)
    out = nc.declare_dram_parameter("out", [QROWS, D], F32, isOutput=True)

    xth_r = xth.rearrange("(dc p) s -> p dc s", p=P)
    xqt_r = xqt.rearrange("(dc p) s -> p dc s", p=P)
    wq_r = wq.rearrange("(dc p) e -> p dc e", p=P)
    wk_r = wk.rearrange("(dc p) e -> p dc e", p=P)
    wv_r = wv.rearrange("(dc p) e -> p dc e", p=P)
    qidx_r = qidx.rearrange("(t p) -> p t", p=P)

    with TileContext(nc) as tc:
        # The race-detector sim can't model pair-aliased Shared DRAM (it
        # demands a single writer); ordering for the shared exchange is
        # enforced with explicit deps instead.
        tc.race_detector_enabled = False

        # Long-lived tiles. K^T / V are per-key-block so attention only
        # waits on the specific block's collective, not the whole tensor.
        persist = tc.alloc_tile_pool(name="persist", bufs=1)
        qt_sb = persist.tile([P, DC, QROWS], BF16, tag="qt_sb")   # Q^T [e, q]
        kt_b = [
            persist.tile([P, DC, KBLK], BF16, tag=f"kt_b{v}", name=f"kt_b{v}")
            for v in range(NKB)
        ]
        v_b = [
            persist.tile([P, KBLK // P, D], BF16, tag=f"v_b{v}", name=f"v_b{v}")
            for v in range(NKB)
        ]
        kpos_f = persist.tile([P, S], F32, tag="kpos_f")
        qidx_sb = persist.tile([P, QT], F32, tag="qidx_sb")
        ident = persist.tile([P, P], BF16, tag="ident")

        nc.sync.dma_start(qidx_sb[:], qidx_r)
        make_identity(nc, ident[:])

        # ---- Phase 1: projections + pair-wise K/V exchange ----
        with (
            tc.tile_pool(name="proj_in", bufs=1) as proj_in,
            tc.tile_pool(name="proj_w", bufs=2) as proj_w,
            tc.tile_pool(name="proj_st", bufs=2) as proj_st,
            tc.tile_pool(name="proj_ps", bufs=6, space="PSUM") as proj_ps,
            tc.tile_pool(name="cc_dram", bufs=1, space="DRAM") as cc_dram,
        ):
            # Dependency-free throwaway collective, emitted before anything
            # else: absorbs the ~50us (jittery) ncfw boot so the rendezvous
            # barriers below run at their ~5us post-boot cost. Collectives
            # also have a ~23us minimum spacing on this firmware, so the
            # earlier this runs the earlier the real barriers may run.
            # Barrier collectives gather garbage — no input producer needed.
            wm_in = cc_dram.tile([16], F32, tag="wm_in")
            wm_out = cc_dram.tile([2, 16], F32, tag="wm_out")
            nc.gpsimd.collective_compute(
                "AllGather",
                mybir.AluOpType.bypass,
                replica_groups=GROUPS,
                ins=[wm_in[:]],
                outs=[wm_out[:]],
            )

            xth_sb = proj_in.tile([P, DC, SH], BF16, tag="xth_sb")
            xqt_sb = proj_in.tile([P, DC, QROWS], BF16, tag="xqt_sb")

            # iota values < 2048 are exact in f32
            nc.gpsimd.iota(
                kpos_f[:], pattern=[[1, S]], base=0, channel_multiplier=0,
                allow_small_or_imprecise_dtypes=True,
            )

            # All four K/V half-exchanges fit under PE cover when launched
            # in consumption order (KT0, KT1, V0, V1 — V blocks are consumed
            # latest). DMA order = first-use order, with the first-needed
            # tensors split fine so all 16 DMA queues fill immediately.
            # The very first matmul group (KTh0, et=0) needs only wk columns
            # 0:128 plus xth half 0 — ship exactly that first, spread over
            # many queues, so the TensorEngine starts as early as possible.
            wk_sb = proj_w.tile([P, DC, D], BF16, tag="w", bufs=3)
            for dc in range(0, DC, 2):
                nc.sync.dma_start(
                    wk_sb[:, dc : dc + 2, 0:P], wk_r[:, dc : dc + 2, 0:P]
                )
            for dc in range(DC):
                nc.sync.dma_start(
                    xth_sb[:, dc, 0:KBLK], xth_r[:, dc, 0:KBLK]
                )
            for et in range(1, 8):
                esl = slice(et * P, (et + 1) * P)
                for dc in range(0, DC, 4):
                    nc.sync.dma_start(
                        wk_sb[:, dc : dc + 4, esl], wk_r[:, dc : dc + 4, esl]
                    )
            wv_sb = proj_w.tile([P, DC, D], BF16, tag="w", bufs=3)
            for dc in range(0, DC, 2):
                nc.sync.dma_start(wv_sb[:, dc : dc + 2, :], wv_r[:, dc : dc + 2, :])
            for dc in range(0, DC, 2):
                nc.sync.dma_start(
                    xth_sb[:, dc : dc + 2, KBLK:SH], xth_r[:, dc : dc + 2, KBLK:SH]
                )
            wq_sb = proj_w.tile([P, DC, D], BF16, tag="w", bufs=3)
            for dc in range(0, DC, 2):
                nc.sync.dma_start(wq_sb[:, dc : dc + 2, :], wq_r[:, dc : dc + 2, :])
            for dc in range(0, DC, 2):
                nc.sync.dma_start(xqt_sb[:, dc : dc + 2, :], xqt_r[:, dc : dc + 2, :])

            # K^T/V halves are exchanged through pair-shared DRAM (cores
            # 2k/2k+1 alias addr_space="Shared" allocations): each core
            # DMA-writes its stagings into its rank's slot (runtime branch
            # on the rank register — the only non-data-driven role split),
            # one tiny AllGather acts as the pair rendezvous, then both
            # halves are DMA-read back at full bandwidth. This replaces 4
            # slow data collectives (~20us/MB) with plain DMA.
            def v_half(h):
                vst = proj_st.tile(
                    [P, KBLK // P, D], BF16, tag=f"vst{h}", name=f"vst{h}", bufs=1
                )
                for st in range(KBLK // P):
                    for ec in range(D // KBLK):
                        ps = proj_ps.tile([P, KBLK], F32, tag="proj_ps")
                        for dc in range(DC):
                            nc.tensor.matmul(
                                ps[:],
                                xth_sb[:, dc, h * KBLK + st * P : h * KBLK + (st + 1) * P],
                                wv_sb[:, dc, ec * KBLK : (ec + 1) * KBLK],
                                start=(dc == 0),
                                stop=(dc == DC - 1),
                            )
                        nc.scalar.copy(vst[:, st, ec * KBLK : (ec + 1) * KBLK], ps[:])
                return vst

            def kt_half(h):
                ssl = slice(h * KBLK, (h + 1) * KBLK)
                ktst = proj_st.tile(
                    [P, DC, KBLK], BF16, tag=f"ktst{h}", name=f"ktst{h}", bufs=1
                )
                for et in range(DC):
                    ps = proj_ps.tile([P, KBLK], F32, tag="proj_ps")
                    for dc in range(DC):
                        nc.tensor.matmul(
                            ps[:],
                            wk_sb[:, dc, et * P : (et + 1) * P],
                            xth_sb[:, dc, ssl],
                            start=(dc == 0),
                            stop=(dc == DC - 1),
                        )
                    nc.scalar.copy(ktst[:, et, :], ps[:])
                return ktst

            # One Shared tensor per (rank, slot) — the scheduler sim demands
            # a single writer inst per Shared DRAM tensor. Slots: 0=KTh0,
            # 1=Vh0, 2=KTh1, 3=Vh1 (flat 512K bf16 each).
            sh_d = [
                [
                    cc_dram.tile(
                        [D * KBLK], BF16, tag=f"sh_d{r}{j}",
                        name=f"sh_d{r}{j}", addr_space="Shared",
                    )
                    for j in range(4)
                ]
                for r in range(2)
            ]

            def kt_view(flat):
                return flat.rearrange("(et p s) -> p et s", p=P, s=KBLK)

            def v_view(flat):
                return flat.rearrange("(st p e) -> p st e", p=P, e=D)

            rk_reg = nc.sync.alloc_register("rk_reg")
            nc.sync.reg_load(rk_reg, rk[0:1, 0:1])

            def exchange_half(h, ktst, vst):
                # Write my stagings into my rank's pair-shared slots, tiny
                # AllGather as the pair rendezvous, read back both ranks'.
                writes = []
                with tc.If(nc.sync.snap(rk_reg) == 0) as cmp:
                    writes.append(
                        nc.sync.dma_start(kt_view(sh_d[0][2 * h]), ktst[:])
                    )
                    writes.append(
                        nc.sync.dma_start(v_view(sh_d[0][2 * h + 1]), vst[:])
                    )
                with cmp.Else():
                    writes.append(
                        nc.sync.dma_start(kt_view(sh_d[1][2 * h]), ktst[:])
                    )
                    writes.append(
                        nc.sync.dma_start(v_view(sh_d[1][2 * h + 1]), vst[:])
                    )
                b_in = cc_dram.tile([16], F32, tag=f"b_in{h}", name=f"b_in{h}")
                b_out = cc_dram.tile(
                    [2, 16], F32, tag=f"b_out{h}", name=f"b_out{h}"
                )
                cc = nc.gpsimd.collective_compute(
                    "AllGather",
                    mybir.AluOpType.bypass,
                    replica_groups=GROUPS,
                    ins=[b_in[:]],
                    outs=[b_out[:]],
                )
                for w in writes:
                    add_dep_helper(cc.ins, w.ins, True, "barrier after writes")
                for rank in range(2):
                    rd_k = nc.sync.dma_start(
                        kt_b[2 * rank + h][:], kt_view(sh_d[rank][2 * h])
                    )
                    rd_v = nc.sync.dma_start(
                        v_b[2 * rank + h][:], v_view(sh_d[rank][2 * h + 1])
                    )
                    add_dep_helper(rd_k.ins, cc.ins, True, "read after rdv")
                    add_dep_helper(rd_v.ins, cc.ins, True, "read after rdv")

            ktst0 = kt_half(0)
            vst0 = v_half(0)
            exchange_half(0, ktst0, vst0)
            ktst1 = kt_half(1)
            vst1 = v_half(1)
            exchange_half(1, ktst1, vst1)

            # Q^T [e, q] = Wq^T @ xq^T (overlaps the second collective).
            for et in range(DC):
                for sc in range(QROWS // KBLK):
                    ps = proj_ps.tile([P, KBLK], F32, tag="proj_ps")
                    for dc in range(DC):
                        nc.tensor.matmul(
                            ps[:],
                            wq_sb[:, dc, et * P : (et + 1) * P],
                            xqt_sb[:, dc, sc * KBLK : (sc + 1) * KBLK],
                            start=(dc == 0),
                            stop=(dc == DC - 1),
                        )
                    nc.scalar.copy(qt_sb[:, et, sc * KBLK : (sc + 1) * KBLK], ps[:])

        # ---- Phase 2: block attention ----
        with (
            tc.tile_pool(name="att", bufs=2) as att,
            tc.tile_pool(name="att_sm", bufs=3) as att_sm,
            tc.tile_pool(name="ps_sc", bufs=2, space="PSUM") as ps_sc,
            tc.tile_pool(name="ps_pt", bufs=2, space="PSUM") as ps_pt,
            tc.tile_pool(name="ps_ctx", bufs=2, space="PSUM") as ps_ctx,
        ):
            for qt in range(QT):
                # 256-row slot s = qt//2 visits s+1 key blocks.
                nvis = qt // 2 + 1
                nkc = nvis * (KBLK // P)
                p_sb = att.tile([P, S], BF16, tag="p_sb")
                pt_sb = att.tile([P, S // P, P], BF16, tag="pt_sb")
                sums = att_sm.tile([P, NKB], F32, tag="sums")
                qcol = qidx_sb[:, qt : qt + 1]

                for v in range(nvis):
                    ksl = slice(v * KBLK, (v + 1) * KBLK)
                    sc_ps = ps_sc.tile([P, KBLK], F32, tag="sc_ps")
                    for ec in range(DC):
                        nc.tensor.matmul(
                            sc_ps[:],
                            qt_sb[:, ec, qt * P : (qt + 1) * P],
                            kt_b[v][:, ec, :],
                            start=(ec == 0),
                            stop=(ec == DC - 1),
                        )
                    bias = att_sm.tile([P, KBLK], F32, tag="bias")
                    nc.vector.tensor_scalar(
                        bias[:], kpos_f[:, ksl], qcol, MASK_NEG,
                        mybir.AluOpType.is_gt, mybir.AluOpType.mult,
                    )
                    sm = att_sm.tile([P, KBLK], F32, tag="sm")
                    nc.vector.tensor_add(sm[:], sc_ps[:], bias[:])
                    nc.scalar.activation(
                        p_sb[:, ksl], sm[:],
                        mybir.ActivationFunctionType.Exp,
                        scale=float(SCALE),
                        accum_out=sums[:, v : v + 1],
                    )

                for kc in range(nkc):
                    pt_ps = ps_pt.tile([P, P], BF16, tag="pt_ps")
                    nc.tensor.transpose(
                        pt_ps[:], p_sb[:, kc * P : (kc + 1) * P], ident[:]
                    )
                    nc.vector.tensor_copy(pt_sb[:, kc, :], pt_ps[:])

                tot = att_sm.tile([P, 1], F32, tag="tot")
                rinv = att_sm.tile([P, 1], F32, tag="rinv")
                nc.vector.reduce_sum(
                    tot[:], sums[:, :nvis], axis=mybir.AxisListType.X
                )
                nc.vector.reciprocal(rinv[:], tot[:])

                ctx_lo = ps_ctx.tile([P, KBLK], F32, tag="ctx_lo")
                ctx_hi = ps_ctx.tile([P, KBLK], F32, tag="ctx_hi")
                for kc in range(nkc):
                    vb = v_b[kc // (KBLK // P)]
                    vrow = kc % (KBLK // P)
                    nc.tensor.matmul(
                        ctx_lo[:], pt_sb[:, kc, :], vb[:, vrow, 0:KBLK],
                        start=(kc == 0), stop=(kc == nkc - 1),
                    )
                    nc.tensor.matmul(
                        ctx_hi[:], pt_sb[:, kc, :], vb[:, vrow, KBLK:D],
                        start=(kc == 0), stop=(kc == nkc - 1),
                    )

                out_sb = att.tile([P, D], F32, tag="out_sb")
                nc.vector.tensor_scalar_mul(out_sb[:, 0:KBLK], ctx_lo[:], rinv[:])
                nc.vector.tensor_scalar_mul(out_sb[:, KBLK:D], ctx_hi[:], rinv[:])
                nc.sync.dma_start(out[qt * P : (qt + 1) * P, :], out_sb[:])

        persist.release()

    return _split_multi_waits(nc)


_NC_CACHE = None


def _get_nc():
    global _NC_CACHE
    if _NC_CACHE is None:
        _NC_CACHE = _build_nc()
    return _NC_CACHE


_TILE256 = {0: (0, 3, 4, 7), 1: (1, 2, 5, 6)}


def _qrows(role):
    # 256-row tiles ordered by ascending visit-need (1,2,3,4 key blocks).
    return np.concatenate(
        [np.arange(t * 256, (t + 1) * 256) for t in _TILE256[role]]
    )


def _shard_inputs(x, Wq, Wk, Wv):
    bf = ml_dtypes.bfloat16
    w = {
        "wq": np.ascontiguousarray(Wq.astype(bf)),
        "wk": np.ascontiguousarray(Wk.astype(bf)),
        "wv": np.ascontiguousarray(Wv.astype(bf)),
    }
    in_maps = []
    for c in range(NCORES):
        b, r = c // 2, c % 2
        rows = _qrows(r)
        xbT = x[b].T.astype(bf)                                  # [D, S]
        in_maps.append(
            {
                "xth": np.ascontiguousarray(xbT[:, r * SH : (r + 1) * SH]),
                "xqt": np.ascontiguousarray(xbT[:, rows]),
                "qidx": rows.astype(np.float32),
                "rk": np.array([[r]], dtype=np.uint32),
                **w,
            }
        )
    return in_maps


def _unshard(results, dtype):
    out = np.empty((B, S, D), dtype=dtype)
    for c in range(NCORES):
        b, r = c // 2, c % 2
        out[b, _qrows(r), :] = results[c]["out"]
    return out


def run(x, Wq, Wk, Wv, trace=False, tmpdir=None):
    from concourse.bass_utils import run_bass_kernel_spmd

    nc = _get_nc()
    in_maps = _shard_inputs(x, Wq, Wk, Wv)
    res = run_bass_kernel_spmd(
        nc, in_maps, core_ids=list(range(NCORES)), trace=trace, tmpdir=tmpdir
    )
    return _unshard(res.results, np.dtype(x.dtype)), res


def kernel(x, Wq, Wk, Wv):
    out, _ = run(np.asarray(x), np.asarray(Wq), np.asarray(Wk), np.asarray(Wv))
    return out
